# revision 34
# baseline (speedup 1.0000x reference)
"""Trainium2 Bass kernel for multi-head attention (B=4, N=2048, C=768, H=12).

Sharding: zero-collective data parallel across 8 NeuronCores. Core c handles
batch b=c//2 and query rows [(c%2)*1024, +1024). K/V are computed over all
2048 keys of the batch (softmax over keys is permutation invariant, so each
core receives its x with its own query rows rolled to the front). Pairs of
cores duplicate only the K/V projection; there are no collectives.

v4: all matmul operands fp16 (1 PE pass/col vs f32r's LOW_HIGH 2-pass; PSUM
stays f32), K resident in SBUF (no DRAM staging), P2 emitted per ~1.3us slot:
  [ S~^T mm x2 (head h) | S-max mm x2 (head h+2) | AV mm x2 (head h, lag 1) ]
so the PE stream never stalls (a stall also drops PE 2.4 -> 1.2 GHz pstate).
The max pass runs TWO heads ahead (its transposing DRAM bounce into q row 64
is hidden by a full head); head 0's rides inside P1's v-projection loop and
head 1's right after P1. exp() and the row-max reduce both work on
[128,1024] PSUM tiles (ACT/DVE instruction overhead amortized); S^T and
S-max tiles share one 3-buffer PSUM pool (6 banks) + 2x[65,512] AV = 8 banks.
Denominators: ACT copies AV PSUM->SBUF (frees the accumulators fast), DVE
reciprocal_approx_fast (~18 bits, den is O(1..2048) thanks to the max shift),
GpSimd partition-broadcast + multiply (GpSimd cannot touch PSUM, SBUF only).
"""

import numpy as np

NQ = 1024  # queries per core
NK = 2048  # keys per core
CD = 768
H = 12
D = 64
P = 128

_CACHE = {}


def _build():
    from contextlib import ExitStack

    import concourse.bacc as bacc
    import concourse.mybir as mybir
    import concourse.tile as tile

    f32 = mybir.dt.float32
    f16 = mybir.dt.float16
    EXP = mybir.ActivationFunctionType.Exp
    AX = mybir.AxisListType.X

    nc = bacc.Bacc("TRN2", target_bir_lowering=False, debug=False, num_devices=8)

    x_ap = nc.dram_tensor("x", [NK, CD], f32, kind="ExternalInput").ap()
    wqkv_ap = nc.dram_tensor("w_qkv", [CD, 3 * CD], f32, kind="ExternalInput").ap()
    wproj_ap = nc.dram_tensor("w_proj", [CD, CD], f32, kind="ExternalInput").ap()
    ident_ap = nc.dram_tensor("ident", [P, P], f32, kind="ExternalInput").ap()
    # crow[1,:] = -1.0 (k-tilde row 64: applies the -max shift in the S~^T mm)
    crow_ap = nc.dram_tensor("crow", [2, NK], f16, kind="ExternalInput").ap()
    out_ap = nc.dram_tensor("out", [NQ, CD], f32, kind="ExternalOutput").ap()

    mstg_ap = nc.dram_tensor("mstg", [H, NQ], f16).ap()  # max transpose bounce

    x_t = x_ap.rearrange("(t p) c -> p t c", p=P)  # [128, 16, 768]
    wqkv_t = wqkv_ap.rearrange("(a p) n -> p a n", p=P)  # [128, 6, 2304]
    wproj_t = wproj_ap.rearrange("(a p) n -> p a n", p=P)  # [128, 6, 768]
    out_t = out_ap.rearrange("(t p) c -> p t c", p=P)  # [128, 8, 768]

    with tile.TileContext(nc) as tc, ExitStack() as ctx:
        pers = ctx.enter_context(tc.tile_pool(name="pers", bufs=1))
        # padded so downstream pools start 4KB-aligned (moving-operand rule)
        ident_sb = pers.tile([P, P], f32, tag="ident", padded_shape=[P, 1024])
        nc.sync.dma_start(ident_sb[:], ident_ap)

        # persistent q/k/v (all fp16): q 12x2KB, k 12x4KB, vhat 25KB per part
        qpool = ctx.enter_context(tc.tile_pool(name="qt", bufs=1))
        q_tiles = [
            qpool.tile([D + 1, NQ], f16, tag=f"q{h}", name=f"q{h}") for h in range(H)
        ]
        kpool = ctx.enter_context(tc.tile_pool(name="kt", bufs=1))
        k_tiles = [
            kpool.tile([D + 1, NK], f16, tag=f"k{h}", name=f"k{h}") for h in range(H)
        ]
        vpool = ctx.enter_context(tc.tile_pool(name="vhat", bufs=1))
        vhat = vpool.tile([P, 16, H, D + 1], f16, tag="vhat")

        mxp = ctx.enter_context(tc.tile_pool(name="mx", bufs=3))

        # S-max machinery. Pool is passed per call: psS0 (own 4 banks) during
        # the P1 era for heads 0/1, the shared psX pool in steady state.
        st = {}

        def start_S(h, width):
            # width = reduce granularity: 2 slots per mx2 col (P1-era pools,
            # [128,1024] pair tiles) or 1 (P2 steady, [128,512] tiles)
            st["mx2", h] = mxp.tile([P, 32 // width], f32, tag="mx2", name=f"mx2{h}")
            st["mxw", h] = width

        def emit_S_mm(pool, tag, h, s):
            """One S-max matmul (queries qt=s//4, keys kc=s%4, K=64); DVE
            row-max reduce per tile ([128,1024] pair or single [128,512])."""
            qt, kc = s // 4, s % 4
            width = st["mxw", h]
            if s % width == 0:
                st["sps", h] = pool.tile(
                    [P, 512 * width], f32, tag=tag, name=f"s{h}_{s}"
                )
            ps = st["sps", h]
            nc.tensor.matmul(
                ps[:, (s % width) * 512 : (s % width) * 512 + 512],
                q_tiles[h][0:D, qt * P : (qt + 1) * P],
                k_tiles[h][0:D, kc * 512 : (kc + 1) * 512],
                start=True,
                stop=True,
            )
            if s % width == width - 1:
                nc.vector.reduce_max(
                    st["mx2", h][:, s // width : s // width + 1], ps[:], axis=AX
                )

        def finalize_S(h):
            m2 = st.pop(("mx2", h))
            b = 4 // st.pop(("mxw", h))
            m8 = mxp.tile([P, 8], f32, tag="mx8", name=f"mx8{h}")
            nc.vector.reduce_max(
                m8[:], m2[:].rearrange("p (a b) -> p a b", b=b), axis=AX
            )
            mr = mxp.tile([P, 8], f16, tag="mxr", name=f"mxr{h}")
            nc.scalar.copy(mr[:], m8[:])
            # scatter maxes into q row 64 (elem (p,qt) -> col qt*128+p)
            nc.sync.dma_start(mstg_ap[h].rearrange("(a b) -> b a", b=P), mr[:])
            nc.sync.dma_start(q_tiles[h][D : D + 1, 0:NQ], mstg_ap[h])

        ctx_psS0 = ExitStack()
        psS0 = ctx_psS0.enter_context(tc.tile_pool(name="psS0", bufs=2, space="PSUM"))

        ctx_xT = ExitStack()
        xT_pool = ctx_xT.enter_context(tc.tile_pool(name="xT", bufs=1))
        xT = xT_pool.tile([P, 6, NK], f16, tag="xT")  # [C-chunk part, kc, row]

        # ================= P0: load x, transpose =================
        with (
            tc.tile_pool(name="xn", bufs=1) as xnp,
            tc.tile_pool(name="ps0", bufs=4, space="PSUM") as ps0,
        ):
            xn = xnp.tile([P, 16, CD], f32, tag="xn")
            for tq in range(4):
                nc.sync.dma_start(
                    xn[:, 4 * tq : 4 * (tq + 1), :], x_t[:, 4 * tq : 4 * (tq + 1), :]
                )
            for kc in range(6):
                for tq in range(4):
                    pst = ps0.tile([P, 512], f32, tag="tr")
                    for ti in range(4):
                        t = 4 * tq + ti
                        nc.tensor.transpose(
                            pst[:, ti * P : (ti + 1) * P],
                            xn[:, t, kc * P : (kc + 1) * P],
                            ident_sb[:],
                        )
                    # f32 -> f16 round; alternate engines (both idle in P0)
                    eng = nc.scalar.copy if (kc + tq) % 2 else nc.vector.tensor_copy
                    eng(xT[:, kc, tq * 512 : (tq + 1) * 512], pst[:])

        # ================= P1: QKV projections =================
        def qk_col_block(j, wq_r, col_off, is_q):
            """One 128-col block (heads 2j, 2j+1) of the q or k projection."""
            nch = 2 if is_q else 4
            dsts = (
                [q_tiles[2 * j], q_tiles[2 * j + 1]]
                if is_q
                else [k_tiles[2 * j], k_tiles[2 * j + 1]]
            )
            for ncn in range(nch):
                ps = ps1.tile([P, 512], f32, tag="qk")
                for kc in range(6):
                    nc.tensor.matmul(
                        ps[:],
                        wq_r[:, kc, col_off + j * P : col_off + (j + 1) * P],
                        xT[:, kc, ncn * 512 : (ncn + 1) * 512],
                        start=(kc == 0),
                        stop=(kc == 5),
                    )
                # split the two PSUM->SBUF copies across ACT and DVE
                nc.scalar.copy(
                    dsts[0][0:D, ncn * 512 : (ncn + 1) * 512], ps[0:D, :]
                )
                nc.vector.tensor_copy(
                    dsts[1][0:D, ncn * 512 : (ncn + 1) * 512], ps[D : 2 * D, :]
                )
            if not is_q:
                for i in range(2):
                    nc.sync.dma_start(dsts[i][D : D + 1, :], crow_ap[1, :])

        with (
            tc.tile_pool(name="wq", bufs=2) as wqp,
            tc.tile_pool(name="wtmp", bufs=2) as wtp,
            tc.tile_pool(name="ps1", bufs=2, space="PSUM") as ps1,
            tc.tile_pool(name="ps1v", bufs=2, space="PSUM") as ps1v,
        ):
            for half in range(2):
                wq_r = wqp.tile([P, 6, 1152], f16, tag="wq")
                for a in range(6):
                    wtmp = wtp.tile([P, 1152], f32, tag="wt")
                    nc.sync.dma_start(
                        wtmp[:], wqkv_t[:, a, half * 1152 : (half + 1) * 1152]
                    )
                    nc.vector.tensor_copy(wq_r[:, a, :], wtmp[:])
                if half == 0:
                    for j in range(6):
                        qk_col_block(j, wq_r, 0, is_q=True)  # q cols 0..767
                    for j in range(3):
                        qk_col_block(j, wq_r, 768, is_q=False)  # k cols 768..1151
                else:
                    # head 1's S-max pass rides inside the half-1 k blocks
                    # (its q/k tiles were written in half 0)
                    start_S(1, 2)
                    s1 = iter(range(32))
                    for j in range(3, 6):
                        # k col block j lives at local offset j*128 - 384 in half 1
                        qk_col_block(j, wq_r, -384, is_q=False)
                        for _ in range(11):
                            s = next(s1, None)
                            if s is not None:
                                emit_S_mm(psS0, "S", 1, s)
                    finalize_S(1)
                    # v projection: cols 1536..2303 -> half offset 384..1152
                    # (head 0's S-max pass rides along, one mm per iteration)
                    start_S(0, 2)
                    for t in range(16):
                        for vc in range(2):
                            ps = ps1v.tile([P, 384], f32, tag="vp")
                            for kc in range(6):
                                nc.tensor.matmul(
                                    ps[:],
                                    xT[:, kc, t * P : (t + 1) * P],
                                    wq_r[:, kc, 384 + vc * 384 : 384 + (vc + 1) * 384],
                                    start=(kc == 0),
                                    stop=(kc == 5),
                                )
                            nc.scalar.copy(
                                vhat[:, t, 6 * vc : 6 * (vc + 1), 0:D],
                                ps[:].rearrange("p (h d) -> p h d", d=D),
                            )
                            emit_S_mm(psS0, "S", 0, 2 * t + vc)
                    finalize_S(0)
            nc.gpsimd.memset(vhat[:, :, :, D], 1.0)
        ctx_xT.close()  # xT dead after P1
        ctx_psS0.close()

        # w_proj load + f16 cast early so the P3 tail starts immediately
        wpp = ctx.enter_context(tc.tile_pool(name="wp", bufs=1))
        wp_r = wpp.tile([P, 6, CD], f16, tag="wp")
        with tc.tile_pool(name="wtmp3", bufs=2) as wtp3:
            for a in range(6):
                wtmp3t = wtp3.tile([P, CD], f32, tag="wt3")
                nc.sync.dma_start(wtmp3t[:], wproj_t[:, a, :])
                nc.vector.tensor_copy(wp_r[:, a, :], wtmp3t[:])
        opool = ctx.enter_context(tc.tile_pool(name="outNT", bufs=1))
        outNT = opool.tile([P, 6, NQ], f16, tag="outNT")

        # ================= P2: attention, 3-stream pipeline =================
        with (
            tc.tile_pool(name="at", bufs=10) as atp,
            tc.tile_pool(name="avstg", bufs=2) as avsp,
            tc.tile_pool(name="den", bufs=2) as denp,
            tc.tile_pool(name="rcp", bufs=2) as rcpp,
            tc.tile_pool(name="rb", bufs=2) as rbp,
            tc.tile_pool(name="psX", bufs=3, space="PSUM") as psX,
            tc.tile_pool(name="psAV", bufs=2, space="PSUM") as psAV,
        ):

            def tail_slot(h, j):
                """S~^T for key chunk j (both query halves) + exp -> at tile."""
                ps_st = psX.tile([P, NQ], f32, tag="x", name=f"st{h}_{j}")
                for ncn in range(2):
                    nc.tensor.matmul(
                        ps_st[:, ncn * 512 : (ncn + 1) * 512],
                        k_tiles[h][:, j * P : (j + 1) * P],
                        q_tiles[h][:, ncn * 512 : (ncn + 1) * 512],
                        start=True,
                        stop=True,
                    )
                at = atp.tile([P, NQ], f16, tag="at")
                nc.scalar.activation(at[:], ps_st[:], EXP, scale=8.0)
                st["at", h, j] = at

            def emit_AV(h, j):
                at = st.pop(("at", h, j))
                if j == 0:
                    st["av", h] = [
                        psAV.tile([D + 1, 512], f32, tag="av", name=f"av{h}n{i}")
                        for i in range(2)
                    ]
                avs = st["av", h]
                for ncn in range(2):
                    nc.tensor.matmul(
                        avs[ncn][:],
                        vhat[:, j, h, :],
                        at[:, ncn * 512 : (ncn + 1) * 512],
                        start=(j == 0),
                        stop=(j == 15),
                    )

            def emit_norm(h):
                """Drain the AV accumulators: numerators and denominators go
                to separate base-partition-0 SBUF tiles (custom DVE ops and
                TensorTensor require matching input base partitions), then
                1/den via approx-fast recip, broadcast + multiply on GpSimd."""
                avs = st.pop(("av", h))
                r0 = D * (h % 2)
                stg = avsp.tile([D, NQ], f32, tag="avstg")
                den = denp.tile([1, NQ], f32, tag="den")
                nc.scalar.copy(stg[:, 0:512], avs[0][0:D, :])
                nc.vector.tensor_copy(stg[:, 512:1024], avs[1][0:D, :])
                nc.scalar.copy(den[:, 0:512], avs[0][D : D + 1, :])
                nc.scalar.copy(den[:, 512:1024], avs[1][D : D + 1, :])
                # (norm tail has ~a head of slack: outNT is only read in P3)
                rs = rcpp.tile([1, NQ], f32, tag="rcp")
                nc.vector.reciprocal_approx_fast(rs[:], den[:])
                rb = rbp.tile([D, NQ], f32, tag="rb")
                nc.gpsimd.partition_broadcast(rb[:], rs[:])
                nc.gpsimd.tensor_tensor(
                    out=outNT[r0 : r0 + D, h // 2, :],
                    in0=stg[:],
                    in1=rb[:],
                    op=mybir.AluOpType.mult,
                )

            # steady state: tail(h) + S-max(h+2), AV lags two slots (so the
            # AV mm never waits on exp: its at tile is a full slot old)
            for h in range(H):
                if h + 2 < H:
                    start_S(h + 2, 2)
                for j in range(16):
                    if h >= 1 and j <= 1:
                        emit_AV(h - 1, 14 + j)
                        if j == 1:
                            emit_norm(h - 1)
                    tail_slot(h, j)
                    if h + 2 < H:
                        emit_S_mm(psX, "x", h + 2, 2 * j)
                        emit_S_mm(psX, "x", h + 2, 2 * j + 1)
                    if j >= 2:
                        emit_AV(h, j - 2)
                if h + 2 < H:
                    finalize_S(h + 2)
            emit_AV(H - 1, 14)
            emit_AV(H - 1, 15)
            emit_norm(H - 1)

        # ================= P3: output projection =================
        with (
            tc.tile_pool(name="ob", bufs=2) as obp,
            tc.tile_pool(name="psP", bufs=2, space="PSUM") as psP,
        ):
            for nt in range(8):
                ps = psP.tile([P, CD], f32, tag="pj")
                for c0, cn in ((0, 512), (512, 256)):
                    for kc in range(6):
                        nc.tensor.matmul(
                            ps[:, c0 : c0 + cn],
                            outNT[:, kc, nt * P : (nt + 1) * P],
                            wp_r[:, kc, c0 : c0 + cn],
                            start=(kc == 0),
                            stop=(kc == 5),
                        )
                osb = obp.tile([P, CD], f32, tag="ob")
                nc.scalar.copy(osb[:], ps[:])
                nc.sync.dma_start(out_t[:, nt, :], osb[:])

    nc.compile()
    return nc


def _in_maps(x, w_qkv, w_proj):
    ident = np.eye(P, dtype=np.float32)
    crow = np.zeros((2, NK), np.float16)
    crow[1, :] = -1.0
    maps = []
    for c in range(8):
        b, qh = c // 2, c % 2
        xb = np.roll(x[b], -qh * NQ, axis=0) if qh else x[b]
        maps.append(
            {
                "x": np.ascontiguousarray(xb, dtype=np.float32),
                "w_qkv": np.ascontiguousarray(w_qkv, dtype=np.float32),
                "w_proj": np.ascontiguousarray(w_proj, dtype=np.float32),
                "ident": ident,
                "crow": crow,
            }
        )
    return maps


LAST = {}


def kernel(x, w_qkv, w_proj):
    import os

    from concourse import bass_utils

    if "nc" not in _CACHE:
        _CACHE["nc"] = _build()
    nc = _CACHE["nc"]
    kwargs = {}
    if os.environ.get("KERNEL_TRACE"):
        kwargs["trace"] = True
        if os.environ.get("KERNEL_TRACE_DIR"):
            kwargs["tmpdir"] = os.environ["KERNEL_TRACE_DIR"]
    res = bass_utils.run_bass_kernel_spmd(
        nc, _in_maps(x, w_qkv, w_proj), core_ids=list(range(8)), **kwargs
    )
    LAST["exec_time_ns"] = res.exec_time_ns
    out = np.empty((4, 2048, CD), np.float32)
    for c in range(8):
        b, qh = c // 2, c % 2
        out[b, qh * NQ : (qh + 1) * NQ] = res.results[c]["out"]
    return out
